# revision 15
# baseline (speedup 1.0000x reference)
"""Additive attention (Bahdanau-style) on 8 TRN2 NeuronCores.

Reference computation (S=1024, B=64, QK=H=DV=1024):
    q = queries @ W_q.T          [S,B,H]
    k = keys    @ W_k.T          [S,B,H]
    f = tanh(q + k)              [S,B,H]
    scores = f @ w_v.T           [S,B,1]
    attn = softmax(scores, axis=S)
    out[b,d] = sum_s attn[s,b] * values[s,0,d]    -> [B,DV]

Strategy: pure data parallel over B (8 batches per core), weights/values
replicated, no collectives.  Per core the dominant work is one fused
matmul [S*BL, 2*QK] @ [2*QK, H] (queries/keys concatenated along the
contraction dim), computed in bf16 with the contraction dim
pre-transposed onto SBUF partitions host-side.  z is produced in
[h, sb] layout (weights stationary); tanh runs on the scalar engine out
of PSUM; scores are produced directly transposed ([sb,1] columns) by
f-stationary matmuls against the w_v column, so exp'd scores land in
the [s, b] layout the final attn^T @ V matmul needs with no shuffle.
Softmax normalization folds into the output copy as a per-partition
1/den scale.

This walrus build rejects engine instructions with 2 embedded sync
waits, so the kernel keeps every ACT/PE instruction at <=1 wait:
tiny ACT "observer" copies advance the scalar engine's vector clock
over ACT-side WAW hazards before each PSUM-consuming activation, and
dummy PE matmuls pre-observe the weight-DMA semaphores.
"""

import numpy as np
import ml_dtypes

P = 128
CORES = 8

FULL_S, FULL_B, FULL_QK, FULL_H, FULL_DV = 1024, 64, 1024, 1024, 1024


def build_nc(S=FULL_S, BL=FULL_B // CORES, QK2=2 * FULL_QK, H=FULL_H,
             DV=FULL_DV, CW=512, XT_BUFS=4, Z_BUFS=3, use_observers=False):
    """Build the single-core Bacc program (same NEFF runs SPMD on all cores)."""
    import concourse.bass as bass
    import concourse.mybir as mybir
    import concourse.tile as tile
    from concourse import bacc

    dt = mybir.dt
    f32, bf16 = dt.float32, dt.bfloat16
    AF = mybir.ActivationFunctionType

    SB = S * BL          # tokens, b-major: sb = b*S + s
    KO = QK2 // P        # contraction subtiles
    HT = H // P          # h tiles
    CH = SB // CW        # token chunks (each chunk: one b, CW s-values)
    SBLK = S // P        # s blocks (final matmul contraction tiles)
    OCW = min(CW, DV)    # output free-dim chunk
    DT = DV // OCW
    TPC = CW // P        # transposed score sub-blocks per chunk
    KQ = max(1, KO // 4)  # xt DMA split granularity
    assert SB % CW == 0 and QK2 % P == 0 and H % P == 0 and S % P == 0
    assert S % CW == 0 and CW % P == 0

    nc = bacc.Bacc("TRN2", debug=False, target_bir_lowering=False)

    xt = nc.dram_tensor("xt", [QK2, SB], bf16, kind="ExternalInput").ap()
    wct = nc.dram_tensor("wct", [QK2, H], bf16, kind="ExternalInput").ap()
    # wv columns per h-tile, plus a trailing all-ones column
    wv = nc.dram_tensor("wv", [P, HT + 1], bf16, kind="ExternalInput").ap()
    vals = nc.dram_tensor("vals", [S, DV], bf16, kind="ExternalInput").ap()
    out = nc.dram_tensor("out", [BL, DV], f32, kind="ExternalOutput").ap()

    xt_t = xt.rearrange("(ko p) n -> p ko n", p=P)      # [P, KO, SB]
    wct_t = wct.rearrange("(ko p) h -> p ko h", p=P)    # [P, KO, H]
    vals_t = vals.rearrange("(so p) d -> p so d", p=P)  # [P, SBLK, DV]

    with tile.TileContext(nc) as tc:
        with tc.tile_pool(name="const", bufs=1) as const_pool, \
             tc.tile_pool(name="xtp", bufs=XT_BUFS) as xt_pool, \
             tc.tile_pool(name="fp", bufs=HT + 2) as f_pool, \
             tc.tile_pool(name="obs", bufs=2) as obs_pool, \
             tc.tile_pool(name="misc", bufs=2) as misc_pool, \
             tc.tile_pool(name="zps", bufs=Z_BUFS, space="PSUM") as zpsum, \
             tc.tile_pool(name="tps", bufs=2, space="PSUM") as tpsum, \
             tc.tile_pool(name="dps", bufs=1, space="PSUM") as dpsum, \
             tc.tile_pool(name="ops", bufs=2, space="PSUM") as opsum:

            # first xt chunk starts streaming before everything else so
            # the PE can begin the z stream as early as possible
            xt_tile0 = xt_pool.tile([P, KO, CW], bf16, tag="xt_tile")
            wct_sb = const_pool.tile([P, KO, H], bf16)
            wv_sb = const_pool.tile([P, HT + 1], bf16)
            for kq in range(0, KO, KQ):
                nc.sync.dma_start(xt_tile0[:, kq:kq + KQ, :],
                                  xt_t[:, kq:kq + KQ, 0:CW])
            nc.sync.dma_start(wct_sb[:, 0, :], wct_t[:, 0, :])
            nc.sync.dma_start(wv_sb[:], wv[:])
            for ko in range(1, KO):
                nc.sync.dma_start(wct_sb[:, ko, :], wct_t[:, ko, :])
            vals_sb = const_pool.tile([P, SBLK, DV], bf16)
            ones_col = wv_sb[:, HT:HT + 1]
            # exp(scores) in [s partitions, sblk, b] layout
            a_sb = const_pool.tile([P, SBLK, BL], bf16)

            # ACT-clock observer: a 1-element copy whose only dep is the
            # previous ACT output, so the following activation needs just
            # its PE wait.
            last_act = [None]

            def act_observe():
                if use_observers and last_act[0] is not None:
                    o = obs_pool.tile([1, 1], f32, tag="obs")
                    nc.scalar.activation(o[:], last_act[0], AF.Copy)

            SH = S // CW
            den_psum = dpsum.tile([BL, 1], f32)
            o_psums = [opsum.tile([BL, OCW], f32, tag="o", name=f"o_psum{d}") for d in range(DT)]

            def final_mms(sblk_range):
                for i, sblk in enumerate(sblk_range):
                    nc.tensor.matmul(
                        den_psum[:], lhsT=a_sb[:, sblk, :], rhs=ones_col,
                        start=(sblk == 0), stop=(sblk == SBLK - 1),
                        skip_group_check=True)
                for d in range(DT):
                    for sblk in sblk_range:
                        nc.tensor.matmul(
                            o_psums[d][:], lhsT=a_sb[:, sblk, :],
                            rhs=vals_sb[:, sblk, d * OCW:(d + 1) * OCW],
                            start=(sblk == 0), stop=(sblk == SBLK - 1),
                            skip_group_check=True)

            for c in range(CH):
                shalf = c // BL
                b = c % BL
                if c == 0:
                    xt_tile = xt_tile0
                else:
                    xt_tile = xt_pool.tile([P, KO, CW], bf16, tag="xt_tile")
                    col = (b * SH + shalf) * CW
                    for kq in range(0, KO, KQ):
                        nc.sync.dma_start(
                            xt_tile[:, kq:kq + KQ, :],
                            xt_t[:, kq:kq + KQ, col:col + CW])
                if c == 2:
                    nc.sync.dma_start(vals_sb[:], vals_t[:])
                if c > 0 and c % BL == 0:
                    # a_sb columns for the previous s-half are complete for
                    # all b: run that slice of the output matmuls now
                    final_mms(range((shalf - 1) * TPC, shalf * TPC))
                f_tiles = []
                for h in range(HT):
                    z_psum = zpsum.tile([P, CW], f32)
                    for ko in range(KO):
                        nc.tensor.matmul(
                            z_psum[:],
                            lhsT=wct_sb[:, ko, h * P:(h + 1) * P],
                            rhs=xt_tile[:, ko, :],
                            start=(ko == 0), stop=(ko == KO - 1))
                    f_tile = f_pool.tile([P, CW], bf16)
                    act_observe()
                    nc.scalar.activation(f_tile[:], z_psum[:], AF.Tanh)
                    last_act[0] = f_tile[0:1, 0:1]
                    f_tiles.append(f_tile)
                for t in range(TPC):
                    tr_psum = tpsum.tile([P, 1], f32, tag="tr")
                    for h in range(HT):
                        nc.tensor.matmul(
                            tr_psum[:],
                            lhsT=f_tiles[h][:, t * P:(t + 1) * P],
                            rhs=wv_sb[:, h:h + 1],
                            start=(h == 0), stop=(h == HT - 1),
                            skip_group_check=True)
                    sblk = shalf * TPC + t
                    act_observe()
                    nc.scalar.activation(a_sb[:, sblk, b:b + 1], tr_psum[:],
                                         AF.Exp)
                    last_act[0] = a_sb[0:1, sblk, b:b + 1]

            final_mms(range((SH - 1) * TPC, SH * TPC))
            den_inv = misc_pool.tile([BL, 1], f32, tag="dinv")
            nc.vector.reciprocal(den_inv[:], den_psum[:])
            # let ACT observe the DVE tick so the scaled output copies
            # carry only their PE wait
            obs_d = obs_pool.tile([1, 1], f32, tag="obs")
            nc.scalar.activation(obs_d[:], den_inv[0:1, 0:1], AF.Copy)

            for d in range(DT):
                o_psum = o_psums[d]
                o_sb = misc_pool.tile([BL, OCW], f32, tag=f"o{d}")
                act_observe()
                nc.scalar.activation(o_sb[:], o_psum[:], AF.Copy,
                                     scale=den_inv[:])
                last_act[0] = o_sb[0:1, 0:1]
                nc.sync.dma_start(out[:, d * OCW:(d + 1) * OCW], o_sb[:])
    return nc


def prep_in_maps(queries, keys, values, W_q, W_k, w_v, n_cores=CORES):
    """Host-side shard + transpose (b-major) + bf16 cast."""
    bf = ml_dtypes.bfloat16
    queries = np.asarray(queries, dtype=np.float32)
    keys = np.asarray(keys, dtype=np.float32)
    S, B, QK = queries.shape
    BL = B // n_cores
    H = np.asarray(W_q).shape[0]
    HT = H // P

    q_bf = queries.astype(bf)
    k_bf = keys.astype(bf)
    wct_np = np.ascontiguousarray(
        np.concatenate([np.asarray(W_q, np.float32),
                        np.asarray(W_k, np.float32)], axis=1).T).astype(bf)
    wv_np = np.empty((P, HT + 1), dtype=bf)
    wv_np[:, :HT] = np.asarray(w_v, np.float32).reshape(HT, P).T.astype(bf)
    wv_np[:, HT] = np.float32(1.0)
    vals_np = np.asarray(values, np.float32)[:, 0, :].astype(bf)

    in_maps = []
    for c in range(n_cores):
        # [S, BL, QK] -> [QK, BL, S] -> [QK, BL*S]   (sb = b*S + s)
        qT = np.ascontiguousarray(
            q_bf[:, c * BL:(c + 1) * BL, :].transpose(2, 1, 0)).reshape(QK, S * BL)
        kT = np.ascontiguousarray(
            k_bf[:, c * BL:(c + 1) * BL, :].transpose(2, 1, 0)).reshape(QK, S * BL)
        xt_np = np.concatenate([qT, kT], axis=0)
        in_maps.append({"xt": xt_np, "wct": wct_np, "wv": wv_np,
                        "vals": vals_np})
    return in_maps


_NC_CACHE = {}


def _get_nc():
    if "nc" not in _NC_CACHE:
        nc = build_nc()
        nc.finalize()
        _NC_CACHE["nc"] = nc
    return _NC_CACHE["nc"]


def kernel_with_results(trace=False, **inputs):
    from concourse.bass_utils import run_bass_kernel_spmd
    nc = _get_nc()
    in_maps = prep_in_maps(**inputs)
    res = run_bass_kernel_spmd(nc, in_maps, core_ids=list(range(CORES)),
                               trace=trace)
    out = np.concatenate([np.asarray(res.results[i]["out"], np.float32)
                          for i in range(CORES)], axis=0)
    return out, res


def kernel(**inputs):
    out, _ = kernel_with_results(trace=False, **inputs)
    return out


# revision 20
# speedup vs baseline: 1.0065x; 1.0065x over previous
"""Additive attention (Bahdanau-style) on 8 TRN2 NeuronCores.

Reference computation (S=1024, B=64, QK=H=DV=1024):
    q = queries @ W_q.T          [S,B,H]
    k = keys    @ W_k.T          [S,B,H]
    f = tanh(q + k)              [S,B,H]
    scores = f @ w_v.T           [S,B,1]
    attn = softmax(scores, axis=S)
    out[b,d] = sum_s attn[s,b] * values[s,0,d]    -> [B,DV]

Strategy: pure data parallel over B (8 batches per core), weights/values
replicated, no collectives.  Per core the dominant work is one fused
matmul [S*BL, 2*QK] @ [2*QK, H] (queries/keys concatenated along the
contraction dim), computed in bf16 with the contraction dim
pre-transposed onto SBUF partitions host-side.  z is produced in
[h, sb] layout (weights stationary); tanh runs on the scalar engine out
of PSUM; scores are produced directly transposed ([sb,1] columns) by
f-stationary matmuls against the w_v column, so exp'd scores land in
the [s, b] layout the final attn^T @ V matmul needs with no shuffle.
Softmax normalization folds into the output copy as a per-partition
1/den scale.

This walrus build rejects engine instructions with more than one
embedded sync wait, so the program must be built as bacc.Bacc and run
through Bacc.compile(): its generate_event_semaphores pass splits
excess on_wait entries onto standalone InstEventSemaphore carriers.
(The optional act_observe machinery below predates that discovery and
is off by default.)

Measured on TRN2: ~492 us HW exec (8 cores SPMD), vs a ~465 us
per-core PE-busy floor (445 us z-matmul stream at the bf16 1-cycle/row
rate + ~19 us score/output matmuls) plus ~20 us fixed preamble/drain.
"""

import numpy as np
import ml_dtypes

P = 128
CORES = 8

FULL_S, FULL_B, FULL_QK, FULL_H, FULL_DV = 1024, 64, 1024, 1024, 1024


def build_nc(S=FULL_S, BL=FULL_B // CORES, QK2=2 * FULL_QK, H=FULL_H,
             DV=FULL_DV, CW=512, XT_BUFS=4, Z_BUFS=3, use_observers=False):
    """Build the single-core Bacc program (same NEFF runs SPMD on all cores)."""
    import concourse.bass as bass
    import concourse.mybir as mybir
    import concourse.tile as tile
    from concourse import bacc

    dt = mybir.dt
    f32, bf16 = dt.float32, dt.bfloat16
    AF = mybir.ActivationFunctionType

    SB = S * BL          # tokens, b-major: sb = b*S + s
    KO = QK2 // P        # contraction subtiles
    HT = H // P          # h tiles
    CH = SB // CW        # token chunks (each chunk: one b, CW s-values)
    SBLK = S // P        # s blocks (final matmul contraction tiles)
    OCW = min(CW, DV)    # output free-dim chunk
    DT = DV // OCW
    TPC = CW // P        # transposed score sub-blocks per chunk
    KQ = max(1, KO // 4)  # xt DMA split granularity
    assert SB % CW == 0 and QK2 % P == 0 and H % P == 0 and S % P == 0
    assert S % CW == 0 and CW % P == 0

    nc = bacc.Bacc("TRN2", debug=False, target_bir_lowering=False)

    xt = nc.dram_tensor("xt", [QK2, SB], bf16, kind="ExternalInput").ap()
    wct = nc.dram_tensor("wct", [QK2, H], bf16, kind="ExternalInput").ap()
    # wv columns per h-tile, plus a trailing all-ones column
    wv = nc.dram_tensor("wv", [P, HT + 1], bf16, kind="ExternalInput").ap()
    vals = nc.dram_tensor("vals", [S, DV], bf16, kind="ExternalInput").ap()
    out = nc.dram_tensor("out", [BL, DV], f32, kind="ExternalOutput").ap()

    xt_t = xt.rearrange("(ko p) n -> p ko n", p=P)      # [P, KO, SB]
    wct_t = wct.rearrange("(ko p) h -> p ko h", p=P)    # [P, KO, H]
    vals_t = vals.rearrange("(so p) d -> p so d", p=P)  # [P, SBLK, DV]

    with tile.TileContext(nc) as tc:
        with tc.tile_pool(name="const", bufs=1) as const_pool, \
             tc.tile_pool(name="xtp", bufs=XT_BUFS) as xt_pool, \
             tc.tile_pool(name="fp", bufs=2 * HT + 2) as f_pool, \
             tc.tile_pool(name="obs", bufs=2) as obs_pool, \
             tc.tile_pool(name="misc", bufs=2) as misc_pool, \
             tc.tile_pool(name="zps", bufs=Z_BUFS, space="PSUM") as zpsum, \
             tc.tile_pool(name="tps", bufs=2, space="PSUM") as tpsum, \
             tc.tile_pool(name="dps", bufs=1, space="PSUM") as dpsum, \
             tc.tile_pool(name="ops", bufs=2, space="PSUM") as opsum:

            # first xt chunk starts streaming before everything else so
            # the PE can begin the z stream as early as possible
            xt_tile0 = xt_pool.tile([P, KO, CW], bf16, tag="xt_tile")
            wct_sb = const_pool.tile([P, KO, H], bf16)
            wv_sb = const_pool.tile([P, HT + 1], bf16)
            for kq in range(0, KO, KQ):
                nc.sync.dma_start(xt_tile0[:, kq:kq + KQ, :],
                                  xt_t[:, kq:kq + KQ, 0:CW])
            nc.sync.dma_start(wct_sb[:, 0, :], wct_t[:, 0, :])
            nc.sync.dma_start(wv_sb[:], wv[:])
            for ko in range(1, KO):
                nc.sync.dma_start(wct_sb[:, ko, :], wct_t[:, ko, :])
            vals_sb = const_pool.tile([P, SBLK, DV], bf16)
            ones_col = wv_sb[:, HT:HT + 1]
            # exp(scores) in [s partitions, sblk, b] layout
            a_sb = const_pool.tile([P, SBLK, BL], bf16)

            # ACT-clock observer: a 1-element copy whose only dep is the
            # previous ACT output, so the following activation needs just
            # its PE wait.
            last_act = [None]

            def act_observe():
                if use_observers and last_act[0] is not None:
                    o = obs_pool.tile([1, 1], f32, tag="obs")
                    nc.scalar.activation(o[:], last_act[0], AF.Copy)

            SH = S // CW
            den_psum = dpsum.tile([BL, 1], f32)
            o_psums = [opsum.tile([BL, OCW], f32, tag="o", name=f"o_psum{d}") for d in range(DT)]

            def final_mms(sblk_range):
                for i, sblk in enumerate(sblk_range):
                    nc.tensor.matmul(
                        den_psum[:], lhsT=a_sb[:, sblk, :], rhs=ones_col,
                        start=(sblk == 0), stop=(sblk == SBLK - 1),
                        skip_group_check=True)
                for d in range(DT):
                    for sblk in sblk_range:
                        nc.tensor.matmul(
                            o_psums[d][:], lhsT=a_sb[:, sblk, :],
                            rhs=vals_sb[:, sblk, d * OCW:(d + 1) * OCW],
                            start=(sblk == 0), stop=(sblk == SBLK - 1),
                            skip_group_check=True)

            prev = None

            def emit_scores(f_tiles, shalf, b):
                for t in range(TPC):
                    tr_psum = tpsum.tile([P, 1], f32, tag="tr")
                    for h in range(HT):
                        nc.tensor.matmul(
                            tr_psum[:],
                            lhsT=f_tiles[h][:, t * P:(t + 1) * P],
                            rhs=wv_sb[:, h:h + 1],
                            start=(h == 0), stop=(h == HT - 1),
                            skip_group_check=True)
                    sblk = shalf * TPC + t
                    act_observe()
                    nc.scalar.activation(a_sb[:, sblk, b:b + 1], tr_psum[:],
                                         AF.Exp)
                    last_act[0] = a_sb[0:1, sblk, b:b + 1]

            for c in range(CH):
                shalf = c // BL
                b = c % BL
                if c == 0:
                    xt_tile = xt_tile0
                else:
                    xt_tile = xt_pool.tile([P, KO, CW], bf16, tag="xt_tile")
                    col = (b * SH + shalf) * CW
                    for kq in range(0, KO, KQ):
                        nc.sync.dma_start(
                            xt_tile[:, kq:kq + KQ, :],
                            xt_t[:, kq:kq + KQ, col:col + CW])
                if c == 2:
                    nc.sync.dma_start(vals_sb[:], vals_t[:])
                f_tiles = []
                for h in range(HT):
                    z_psum = zpsum.tile([P, CW], f32)
                    for ko in range(KO):
                        nc.tensor.matmul(
                            z_psum[:],
                            lhsT=wct_sb[:, ko, h * P:(h + 1) * P],
                            rhs=xt_tile[:, ko, :],
                            start=(ko == 0), stop=(ko == KO - 1))
                    f_tile = f_pool.tile([P, CW], bf16)
                    act_observe()
                    nc.scalar.activation(f_tile[:], z_psum[:], AF.Tanh)
                    last_act[0] = f_tile[0:1, 0:1]
                    f_tiles.append(f_tile)
                # score phase pipelined one chunk behind the z stream so
                # the in-order PE never stalls on the current chunk's last
                # tanh (head-of-line blocking at chunk boundaries)
                if prev is not None:
                    emit_scores(*prev)
                prev = (f_tiles, shalf, b)
                if c > 0 and c % BL == 0:
                    # scores for all chunks of the previous s-half are now
                    # emitted: run that slice of the output matmuls
                    final_mms(range((shalf - 1) * TPC, shalf * TPC))

            emit_scores(*prev)
            final_mms(range((SH - 1) * TPC, SH * TPC))
            den_inv = misc_pool.tile([BL, 1], f32, tag="dinv")
            nc.vector.reciprocal(den_inv[:], den_psum[:])
            # let ACT observe the DVE tick so the scaled output copies
            # carry only their PE wait
            obs_d = obs_pool.tile([1, 1], f32, tag="obs")
            nc.scalar.activation(obs_d[:], den_inv[0:1, 0:1], AF.Copy)

            for d in range(DT):
                o_psum = o_psums[d]
                o_sb = misc_pool.tile([BL, OCW], f32, tag=f"o{d}")
                act_observe()
                nc.scalar.activation(o_sb[:], o_psum[:], AF.Copy,
                                     scale=den_inv[:])
                last_act[0] = o_sb[0:1, 0:1]
                nc.sync.dma_start(out[:, d * OCW:(d + 1) * OCW], o_sb[:])
    return nc


def prep_in_maps(queries, keys, values, W_q, W_k, w_v, n_cores=CORES):
    """Host-side shard + transpose (b-major) + bf16 cast."""
    bf = ml_dtypes.bfloat16
    queries = np.asarray(queries, dtype=np.float32)
    keys = np.asarray(keys, dtype=np.float32)
    S, B, QK = queries.shape
    BL = B // n_cores
    H = np.asarray(W_q).shape[0]
    HT = H // P

    q_bf = queries.astype(bf)
    k_bf = keys.astype(bf)
    wct_np = np.ascontiguousarray(
        np.concatenate([np.asarray(W_q, np.float32),
                        np.asarray(W_k, np.float32)], axis=1).T).astype(bf)
    wv_np = np.empty((P, HT + 1), dtype=bf)
    wv_np[:, :HT] = np.asarray(w_v, np.float32).reshape(HT, P).T.astype(bf)
    wv_np[:, HT] = np.float32(1.0)
    vals_np = np.asarray(values, np.float32)[:, 0, :].astype(bf)

    in_maps = []
    for c in range(n_cores):
        # [S, BL, QK] -> [QK, BL, S] -> [QK, BL*S]   (sb = b*S + s)
        qT = np.ascontiguousarray(
            q_bf[:, c * BL:(c + 1) * BL, :].transpose(2, 1, 0)).reshape(QK, S * BL)
        kT = np.ascontiguousarray(
            k_bf[:, c * BL:(c + 1) * BL, :].transpose(2, 1, 0)).reshape(QK, S * BL)
        xt_np = np.concatenate([qT, kT], axis=0)
        in_maps.append({"xt": xt_np, "wct": wct_np, "wv": wv_np,
                        "vals": vals_np})
    return in_maps


_NC_CACHE = {}


def _get_nc():
    if "nc" not in _NC_CACHE:
        nc = build_nc()
        nc.finalize()
        _NC_CACHE["nc"] = nc
    return _NC_CACHE["nc"]


def kernel_with_results(trace=False, **inputs):
    from concourse.bass_utils import run_bass_kernel_spmd
    nc = _get_nc()
    in_maps = prep_in_maps(**inputs)
    res = run_bass_kernel_spmd(nc, in_maps, core_ids=list(range(CORES)),
                               trace=trace)
    out = np.concatenate([np.asarray(res.results[i]["out"], np.float32)
                          for i in range(CORES)], axis=0)
    return out, res


def kernel(**inputs):
    out, _ = kernel_with_results(trace=False, **inputs)
    return out


# revision 26
# speedup vs baseline: 1.0120x; 1.0055x over previous
"""Additive attention (Bahdanau-style) on 8 TRN2 NeuronCores.

Reference computation (S=1024, B=64, QK=H=DV=1024):
    q = queries @ W_q.T          [S,B,H]
    k = keys    @ W_k.T          [S,B,H]
    f = tanh(q + k)              [S,B,H]
    scores = f @ w_v.T           [S,B,1]
    attn = softmax(scores, axis=S)
    out[b,d] = sum_s attn[s,b] * values[s,0,d]    -> [B,DV]

Strategy: pure data parallel over B (8 batches per core), weights/values
replicated, no collectives.  Per core the dominant work is one fused
matmul [S*BL, 2*QK] @ [2*QK, H] (queries/keys concatenated along the
contraction dim), computed in bf16 with the contraction dim
pre-transposed onto SBUF partitions host-side.  z is produced in
[h, sb] layout (weights stationary); tanh runs on the scalar engine out
of PSUM; scores are produced directly transposed ([sb,1] columns) by
f-stationary matmuls against the w_v column, so exp'd scores land in
the [s, b] layout the final attn^T @ V matmul needs with no shuffle.
Softmax normalization folds into the output copy as a per-partition
1/den scale.

This walrus build rejects engine instructions with more than one
embedded sync wait, so the program must be built as bacc.Bacc and run
through Bacc.compile(): its generate_event_semaphores pass splits
excess on_wait entries onto standalone InstEventSemaphore carriers.
(The optional act_observe machinery below predates that discovery and
is off by default.)

Measured on TRN2: ~492 us HW exec (8 cores SPMD), vs a ~465 us
per-core PE-busy floor (445 us z-matmul stream at the bf16 1-cycle/row
rate + ~19 us score/output matmuls) plus ~20 us fixed preamble/drain.
"""

import numpy as np
import ml_dtypes

P = 128
CORES = 8
CHUNK_W = 512   # token-chunk width; build_nc and prep_in_maps must agree

FULL_S, FULL_B, FULL_QK, FULL_H, FULL_DV = 1024, 64, 1024, 1024, 1024


def build_nc(S=FULL_S, BL=FULL_B // CORES, QK2=2 * FULL_QK, H=FULL_H,
             DV=FULL_DV, CW=CHUNK_W, XT_BUFS=4, Z_BUFS=3, use_observers=False):
    """Build the single-core Bacc program (same NEFF runs SPMD on all cores)."""
    import concourse.bass as bass
    import concourse.mybir as mybir
    import concourse.tile as tile
    from concourse import bacc

    dt = mybir.dt
    f32, bf16 = dt.float32, dt.bfloat16
    AF = mybir.ActivationFunctionType

    SB = S * BL          # tokens, b-major: sb = b*S + s
    KO = QK2 // P        # contraction subtiles
    HT = H // P          # h tiles
    CH = SB // CW        # token chunks (each chunk: one b, CW s-values)
    SBLK = S // P        # s blocks (final matmul contraction tiles)
    OCW = min(CW, DV)    # output free-dim chunk
    DT = DV // OCW
    TPC = CW // P        # transposed score sub-blocks per chunk
    KQ = max(1, KO // 4)  # xt DMA split granularity
    assert SB % CW == 0 and QK2 % P == 0 and H % P == 0 and S % P == 0
    assert S % CW == 0 and CW % P == 0

    nc = bacc.Bacc("TRN2", debug=False, target_bir_lowering=False)

    # chunk-major host layouts: each SBUF partition reads one contiguous
    # slab per DMA (minimal descriptor count, full queue bandwidth)
    xt = nc.dram_tensor("xt", [CH, P, KO, CW], bf16, kind="ExternalInput").ap()
    wct = nc.dram_tensor("wct", [P, KO, H], bf16, kind="ExternalInput").ap()
    # wv columns per h-tile, plus a trailing all-ones column
    wv = nc.dram_tensor("wv", [P, HT + 1], bf16, kind="ExternalInput").ap()
    vals = nc.dram_tensor("vals", [P, SBLK, DV], bf16, kind="ExternalInput").ap()
    out = nc.dram_tensor("out", [BL, DV], f32, kind="ExternalOutput").ap()

    with tile.TileContext(nc) as tc:
        with tc.tile_pool(name="const", bufs=1) as const_pool, \
             tc.tile_pool(name="xtp", bufs=XT_BUFS) as xt_pool, \
             tc.tile_pool(name="fp", bufs=2 * HT + 2) as f_pool, \
             tc.tile_pool(name="obs", bufs=2) as obs_pool, \
             tc.tile_pool(name="misc", bufs=2) as misc_pool, \
             tc.tile_pool(name="zps", bufs=Z_BUFS, space="PSUM") as zpsum, \
             tc.tile_pool(name="tps", bufs=2, space="PSUM") as tpsum, \
             tc.tile_pool(name="dps", bufs=1, space="PSUM") as dpsum, \
             tc.tile_pool(name="ops", bufs=2, space="PSUM") as opsum:

            # first xt chunk starts streaming before everything else so
            # the PE can begin the z stream as early as possible
            xt_tile0 = xt_pool.tile([P, KO, CW], bf16, tag="xt_tile")
            wct_sb = const_pool.tile([P, KO, H], bf16)
            wv_sb = const_pool.tile([P, HT + 1], bf16)
            for kq in range(0, KO, KQ):
                nc.sync.dma_start(xt_tile0[:, kq:kq + KQ, :],
                                  xt[0, :, kq:kq + KQ, :])
            nc.sync.dma_start(wct_sb[:, 0, :], wct[:, 0, :])
            nc.sync.dma_start(wv_sb[:], wv[:])
            for ko in range(1, KO):
                nc.sync.dma_start(wct_sb[:, ko, :], wct[:, ko, :])
            vals_sb = const_pool.tile([P, SBLK, DV], bf16)
            ones_col = wv_sb[:, HT:HT + 1]
            # exp(scores) in [s partitions, sblk, b] layout
            a_sb = const_pool.tile([P, SBLK, BL], bf16)

            # ACT-clock observer: a 1-element copy whose only dep is the
            # previous ACT output, so the following activation needs just
            # its PE wait.
            last_act = [None]

            def act_observe():
                if use_observers and last_act[0] is not None:
                    o = obs_pool.tile([1, 1], f32, tag="obs")
                    nc.scalar.activation(o[:], last_act[0], AF.Copy)

            SH = S // CW
            den_psum = dpsum.tile([BL, 1], f32)
            o_psums = [opsum.tile([BL, OCW], f32, tag="o", name=f"o_psum{d}") for d in range(DT)]

            def final_mms(sblk_range):
                for i, sblk in enumerate(sblk_range):
                    nc.tensor.matmul(
                        den_psum[:], lhsT=a_sb[:, sblk, :], rhs=ones_col,
                        start=(sblk == 0), stop=(sblk == SBLK - 1),
                        skip_group_check=True)
                for d in range(DT):
                    for sblk in sblk_range:
                        nc.tensor.matmul(
                            o_psums[d][:], lhsT=a_sb[:, sblk, :],
                            rhs=vals_sb[:, sblk, d * OCW:(d + 1) * OCW],
                            start=(sblk == 0), stop=(sblk == SBLK - 1),
                            skip_group_check=True)

            prev = None

            def emit_scores(f_tiles, shalf, b):
                for t in range(TPC):
                    tr_psum = tpsum.tile([P, 1], f32, tag="tr")
                    for h in range(HT):
                        nc.tensor.matmul(
                            tr_psum[:],
                            lhsT=f_tiles[h][:, t * P:(t + 1) * P],
                            rhs=wv_sb[:, h:h + 1],
                            start=(h == 0), stop=(h == HT - 1),
                            skip_group_check=True)
                    sblk = shalf * TPC + t
                    act_observe()
                    nc.scalar.activation(a_sb[:, sblk, b:b + 1], tr_psum[:],
                                         AF.Exp)
                    last_act[0] = a_sb[0:1, sblk, b:b + 1]

            for c in range(CH):
                shalf = c // BL
                b = c % BL
                if c == 0:
                    xt_tile = xt_tile0
                else:
                    xt_tile = xt_pool.tile([P, KO, CW], bf16, tag="xt_tile")
                    cc = b * SH + shalf
                    for kq in range(0, KO, KQ):
                        nc.sync.dma_start(
                            xt_tile[:, kq:kq + KQ, :],
                            xt[cc, :, kq:kq + KQ, :])
                if c == 2:
                    nc.sync.dma_start(vals_sb[:], vals[:])
                f_tiles = []
                for h in range(HT):
                    z_psum = zpsum.tile([P, CW], f32)
                    for ko in range(KO):
                        nc.tensor.matmul(
                            z_psum[:],
                            lhsT=wct_sb[:, ko, h * P:(h + 1) * P],
                            rhs=xt_tile[:, ko, :],
                            start=(ko == 0), stop=(ko == KO - 1))
                    f_tile = f_pool.tile([P, CW], bf16)
                    act_observe()
                    nc.scalar.activation(f_tile[:], z_psum[:], AF.Tanh)
                    last_act[0] = f_tile[0:1, 0:1]
                    f_tiles.append(f_tile)
                # score phase pipelined one chunk behind the z stream so
                # the in-order PE never stalls on the current chunk's last
                # tanh (head-of-line blocking at chunk boundaries)
                if prev is not None:
                    emit_scores(*prev)
                prev = (f_tiles, shalf, b)
                if c > 0 and c % BL == 0:
                    # scores for all chunks of the previous s-half are now
                    # emitted: run that slice of the output matmuls
                    final_mms(range((shalf - 1) * TPC, shalf * TPC))

            emit_scores(*prev)
            final_mms(range((SH - 1) * TPC, SH * TPC))
            den_inv = misc_pool.tile([BL, 1], f32, tag="dinv")
            nc.vector.reciprocal(den_inv[:], den_psum[:])
            # let ACT observe the DVE tick so the scaled output copies
            # carry only their PE wait
            obs_d = obs_pool.tile([1, 1], f32, tag="obs")
            nc.scalar.activation(obs_d[:], den_inv[0:1, 0:1], AF.Copy)

            for d in range(DT):
                o_psum = o_psums[d]
                o_sb = misc_pool.tile([BL, OCW], f32, tag=f"o{d}")
                act_observe()
                nc.scalar.activation(o_sb[:], o_psum[:], AF.Copy,
                                     scale=den_inv[:])
                last_act[0] = o_sb[0:1, 0:1]
                nc.sync.dma_start(out[:, d * OCW:(d + 1) * OCW], o_sb[:])
    return nc


def prep_in_maps(queries, keys, values, W_q, W_k, w_v, n_cores=CORES):
    """Host-side shard + transpose (b-major) + bf16 cast."""
    bf = ml_dtypes.bfloat16
    queries = np.asarray(queries, dtype=np.float32)
    keys = np.asarray(keys, dtype=np.float32)
    S, B, QK = queries.shape
    BL = B // n_cores
    H = np.asarray(W_q).shape[0]
    HT = H // P

    q_bf = queries.astype(bf)
    k_bf = keys.astype(bf)
    KO = 2 * QK // P
    wct_np = np.ascontiguousarray(
        np.concatenate([np.asarray(W_q, np.float32),
                        np.asarray(W_k, np.float32)], axis=1).T
        .astype(bf).reshape(KO, P, H).transpose(1, 0, 2))
    wv_np = np.empty((P, HT + 1), dtype=bf)
    wv_np[:, :HT] = np.asarray(w_v, np.float32).reshape(HT, P).T.astype(bf)
    wv_np[:, HT] = np.float32(1.0)
    DV = np.asarray(values).shape[2]
    vals_np = np.ascontiguousarray(
        np.asarray(values, np.float32)[:, 0, :].astype(bf)
        .reshape(S // P, P, DV).transpose(1, 0, 2))

    in_maps = []
    for c in range(n_cores):
        # [S, BL, QK] -> [QK, BL, S] -> [QK, BL*S]   (sb = b*S + s)
        qT = np.ascontiguousarray(
            q_bf[:, c * BL:(c + 1) * BL, :].transpose(2, 1, 0)).reshape(QK, S * BL)
        kT = np.ascontiguousarray(
            k_bf[:, c * BL:(c + 1) * BL, :].transpose(2, 1, 0)).reshape(QK, S * BL)
        xt_2d = np.concatenate([qT, kT], axis=0)       # [2QK, BL*S]
        CW = CHUNK_W
        CH = S * BL // CW
        xt_np = np.ascontiguousarray(
            xt_2d.reshape(KO, P, CH, CW).transpose(2, 1, 0, 3))
        in_maps.append({"xt": xt_np, "wct": wct_np, "wv": wv_np,
                        "vals": vals_np})
    return in_maps


_NC_CACHE = {}


def _get_nc():
    if "nc" not in _NC_CACHE:
        nc = build_nc()
        nc.finalize()
        _NC_CACHE["nc"] = nc
    return _NC_CACHE["nc"]


def kernel_with_results(trace=False, **inputs):
    from concourse.bass_utils import run_bass_kernel_spmd
    nc = _get_nc()
    in_maps = prep_in_maps(**inputs)
    res = run_bass_kernel_spmd(nc, in_maps, core_ids=list(range(CORES)),
                               trace=trace)
    out = np.concatenate([np.asarray(res.results[i]["out"], np.float32)
                          for i in range(CORES)], axis=0)
    return out, res


def kernel(**inputs):
    out, _ = kernel_with_results(trace=False, **inputs)
    return out
